# revision 7
# baseline (speedup 1.0000x reference)
"""ConvNearestNeightbor Trainium2 kernel.

out[b, n*C+c, i, j] = max_k |x[b,c,i-r_k,j-c_k] - neighbors[n,c,k]|
over the 9 zero-padded 3x3 shifts (r_k, c_k).

Sharding: 8 cores = 4 batch-groups x 2 num-groups.
Per core: B_LOC=4 batches, N_LOC=16 codebook entries.
Partition layout: (nn in 0..3, c in 0..31) -> 128 partitions, with the
codebook tile index nt in 0..3 selecting n = nt*4+nn.

Per (nt, h) chain over one fp16 padded image tile xpad [128, 4,34,34]:
  - 2-3 of the 9 abs-diff planes are DVE tensor_scalar subtracts at the
    4x fp16 rate (window offsets 4B-aligned: corner k's {0,2,6}), with
    one merged bitwise-and clearing sign bits; the other 6-7 planes are
    ScalarE Abs activations with per-partition bias -nb (ACT rate is
    AP/dtype agnostic).  8 VectorE tensor_tensor max ops fold the 9
    planes, ordered to match ACT plane arrival.
  - batch-half 0 is HWDGE-loaded fp32 (sync+scalar queues, early) and
    cast into xpad's interior by one ACT copy; half 1 is SWDGE
    cast-loaded per batch directly into the interior (gpsimd), hiding
    behind half-0 compute.
  - acc is fp16, DMAed to a fp16 DRAM tensor on alternating sync/gpsimd
    queues; the host casts to fp32.  One fp16 rounding (~2^-11 rel).
"""

import numpy as np

B, C, H, W = 16, 32, 32, 32
NUM = 32
NCORES = 8
BG, NG = 4, 2          # batch groups x num groups
B_LOC = B // BG        # 4
N_LOC = NUM // NG      # 16
NT = N_LOC // 4        # 4 codebook tiles of 4 n each
PH, PW = H + 2, W + 2  # 34 x 34 padded image
HB = B_LOC // 2        # batches per half

_module_cache = {}


def _build_module():
    import concourse.bacc as bacc
    import concourse.mybir as mybir
    import concourse.tile as tile

    dt = mybir.dt
    Alu = mybir.AluOpType
    AF = mybir.ActivationFunctionType

    nc = bacc.Bacc("TRN2", debug=False)
    x = nc.dram_tensor("x", [B_LOC, C, H, W], dt.float32, kind="ExternalInput")
    nb = nc.dram_tensor("neighbors", [N_LOC, C, 9], dt.float32, kind="ExternalInput")
    out = nc.dram_tensor(
        "out", [B_LOC, N_LOC * C, H, W], dt.float16, kind="ExternalOutput"
    )

    offs = []
    for row in (-1, 0, 1):
        for col in (-1, 0, 1):
            offs.append((1 - row, 1 - col))

    with tile.TileContext(nc) as tc:
        with (
            tc.tile_pool(name="const", bufs=1) as cpool,
            tc.tile_pool(name="accp", bufs=4) as apool,
            tc.tile_pool(name="dp", bufs=3) as dpool,
            tc.tile_pool(name="dap", bufs=8) as dapool,
        ):
            # neighbors first: tiny DMA gates nbneg -> first ACT abs
            nbt = cpool.tile([128, NT * 9], dt.float32, tag="nbt")
            nb_src = nb.ap().rearrange("(t nn) c k -> (nn c) t k", nn=4)
            nbt_v = nbt[:].rearrange("p (t k) -> p t k", t=NT)
            nc.sync.dma_start(nbt_v, nb_src)

            x_src = x.ap().rearrange("b c h w -> c b (h w)")

            # half-0 raw fp32 via HWDGE on sync+scalar queues (fast start)
            raw0 = cpool.tile([128, HB * H * W], dt.float32, tag="raw0")
            raw0_v = raw0[:].rearrange("p (b h w) -> p b (h w)", b=HB, h=H, w=W)
            for nn in range(4):
                eng = nc.sync if nn < 2 else nc.scalar
                eng.dma_start(
                    raw0_v[nn * 32 : (nn + 1) * 32], x_src[:, 0:HB]
                )

            # single padded fp16 tile, all 4 batches
            xpt = cpool.tile([128, B_LOC * PH * PW], dt.float16, tag="xpad")
            xp = xpt[:].rearrange("p (b h w) -> p b h w", b=B_LOC, h=PH, w=PW)
            nc.gpsimd.memset(xp[:, :, 0:PH:33, :], 0.0)
            nc.gpsimd.memset(xp[:, :, 1:33, 0:PW:33], 0.0)
            # half-1 interior: SWDGE cast-load per (nn, b)
            for bb in (2, 3):
                for nn in range(4):
                    nc.gpsimd.dma_start(
                        xp[nn * 32 : (nn + 1) * 32, bb, 1 : 1 + H, 1 : 1 + W],
                        x_src[:, bb],
                    )

            # negated neighbors for ACT bias: Abs(x + (-nb))
            nbneg = cpool.tile([128, NT * 9], dt.float32, tag="nbneg")
            nc.scalar.mul(nbneg[:], nbt[:], -1.0)
            # half-0 interior: one ACT cast-copy
            nc.scalar.copy(
                xp[:, 0:HB, 1 : 1 + H, 1 : 1 + W],
                raw0_v.rearrange("p b (h w) -> p b h w", h=H, w=W),
            )

            out_v = out.ap().rearrange("b (t p) h w -> t p b (h w)", t=NT)
            out_qs = [nc.sync, nc.gpsimd]
            qi = [0]

            def chain(nt, h, dve_ks, act_ks, split_out=False):
                bs = {0: (0, HB), 1: (HB, B_LOC), None: (0, B_LOC)}[h]
                nbf = bs[1] - bs[0]
                F = nbf * H * W
                acc = apool.tile([128, F], dt.float16, tag="acc")

                def win(k):
                    a, bcol = offs[k]
                    return xp[:, bs[0] : bs[1], a : a + H, bcol : bcol + W]

                nd = len(dve_ks)
                dtile = dpool.tile([128, nd * F], dt.float16, tag="dve")
                for i, k in enumerate(dve_ks):
                    d_v = dtile[:, i * F : (i + 1) * F].rearrange(
                        "p (b h w) -> p b h w", b=nbf, h=H, w=W
                    )
                    nc.vector.tensor_scalar(
                        d_v, win(k), nbt[:, nt * 9 + k : nt * 9 + k + 1],
                        None, Alu.subtract,
                    )
                nc.vector.tensor_scalar(
                    dtile[:].bitcast(dt.uint16), dtile[:].bitcast(dt.uint16),
                    0x7FFF, None, Alu.bitwise_and,
                )
                nc.vector.tensor_tensor(
                    acc[:], dtile[:, 0:F], dtile[:, F : 2 * F], Alu.max
                )
                for i in range(2, nd):
                    nc.vector.tensor_tensor(
                        acc[:], acc[:], dtile[:, i * F : (i + 1) * F], Alu.max
                    )
                for k in act_ks:
                    da = dapool.tile([128, F], dt.float16, tag="dact")
                    da_v = da[:].rearrange("p (b h w) -> p b h w", b=nbf, h=H, w=W)
                    nc.scalar.activation(
                        da_v, win(k), AF.Abs,
                        bias=nbneg[:, nt * 9 + k : nt * 9 + k + 1], scale=1.0,
                    )
                    nc.vector.tensor_tensor(acc[:], acc[:], da[:], Alu.max)

                acc_s = acc[:].rearrange("p (b s) -> p b s", b=nbf)
                if split_out:
                    for bi in range(nbf):
                        out_qs[qi[0] % 2].dma_start(
                            out_v[nt][:, bs[0] + bi : bs[0] + bi + 1],
                            acc_s[:, bi : bi + 1],
                        )
                        qi[0] += 1
                else:
                    dst = out_v[nt] if h is None else out_v[nt][:, bs[0] : bs[1]]
                    out_qs[qi[0] % 2].dma_start(dst, acc_s)
                    qi[0] += 1

            KD3 = (0, 2, 6)
            KD2 = (0, 6)
            KA6 = (4, 1, 3, 5, 7, 8)
            KA7 = (4, 2, 1, 3, 5, 7, 8)
            chain_specs = [
                (0, 0, KD3, KA6, False),
                (3, 0, KD3, KA6, False),
                (1, None, KD3, KA6, False),
                (2, None, KD2, KA7, False),
                (0, 1, KD3, KA6, False),
                (3, 1, KD3, KA6, True),
            ]
            for nt, h, kd, ka, so in chain_specs:
                chain(nt, h, kd, ka, split_out=so)

    nc.compile()
    return nc


def _get_module():
    if "nc" not in _module_cache:
        _module_cache["nc"] = _build_module()
    return _module_cache["nc"]


def _run(x, neighbors, trace=False):
    from concourse import bass_utils

    x = np.ascontiguousarray(x, dtype=np.float32)
    neighbors = np.ascontiguousarray(neighbors, dtype=np.float32)
    in_maps = []
    for core in range(NCORES):
        bg, ng = divmod(core, NG)
        in_maps.append(
            {
                "x": x[bg * B_LOC : (bg + 1) * B_LOC],
                "neighbors": neighbors[ng * N_LOC : (ng + 1) * N_LOC],
            }
        )
    res = bass_utils.run_bass_kernel_spmd(
        _get_module(), in_maps, core_ids=list(range(NCORES)), trace=trace
    )
    out = np.empty((B, NUM * C, H, W), dtype=np.float32)
    for core in range(NCORES):
        bg, ng = divmod(core, NG)
        out[bg * B_LOC : (bg + 1) * B_LOC, ng * N_LOC * C : (ng + 1) * N_LOC * C] = (
            res.results[core]["out"].astype(np.float32)
        )
    return out, res


def kernel(x, neighbors):
    out, _ = _run(x, neighbors, trace=False)
    return out


# revision 10
# speedup vs baseline: 1.0186x; 1.0186x over previous
"""ConvNearestNeightbor Trainium2 kernel.

out[b, n*C+c, i, j] = max_k |x[b,c,i-r_k,j-c_k] - neighbors[n,c,k]|
over the 9 zero-padded 3x3 shifts (r_k, c_k).

Sharding: 8 cores = 4 batch-groups x 2 num-groups.
Per core: B_LOC=4 batches, N_LOC=16 codebook entries.
Partition layout: (nn in 0..3, c in 0..31) -> 128 partitions, with the
codebook tile index nt in 0..3 selecting n = nt*4+nn.

Per (nt, h) chain over one fp16 padded image tile xpad [128, 4,34,34]:
  - 2-3 of the 9 abs-diff planes are DVE tensor_scalar subtracts at the
    4x fp16 rate (window offsets 4B-aligned: corner k's {0,2,6}), with
    one merged bitwise-and clearing sign bits; the other 6-7 planes are
    ScalarE Abs activations with per-partition bias -nb (ACT rate is
    AP/dtype agnostic).  8 VectorE tensor_tensor max ops fold the 9
    planes, ordered to match ACT plane arrival.
  - batch-half 0 is HWDGE-loaded fp32 (sync+scalar queues, early) and
    cast into xpad's interior by one ACT copy; half 1 is SWDGE
    cast-loaded per batch directly into the interior (gpsimd), hiding
    behind half-0 compute.
  - acc is fp16, DMAed to a fp16 DRAM tensor on alternating sync/gpsimd
    queues; the host casts to fp32.  One fp16 rounding (~2^-11 rel).
"""

import numpy as np

B, C, H, W = 16, 32, 32, 32
NUM = 32
NCORES = 8
BG, NG = 4, 2          # batch groups x num groups
B_LOC = B // BG        # 4
N_LOC = NUM // NG      # 16
NT = N_LOC // 4        # 4 codebook tiles of 4 n each
PH, PW = H + 2, W + 2  # 34 x 34 padded image
HB = B_LOC // 2        # batches per half

_module_cache = {}


def _build_module():
    import concourse.bacc as bacc
    import concourse.mybir as mybir
    import concourse.tile as tile

    dt = mybir.dt
    Alu = mybir.AluOpType
    AF = mybir.ActivationFunctionType

    nc = bacc.Bacc("TRN2", debug=False)
    x = nc.dram_tensor("x", [B_LOC, C, H, W], dt.float32, kind="ExternalInput")
    nb = nc.dram_tensor("neighbors", [N_LOC, C, 9], dt.float32, kind="ExternalInput")
    out = nc.dram_tensor(
        "out", [B_LOC, N_LOC * C, H, W], dt.float16, kind="ExternalOutput"
    )

    offs = []
    for row in (-1, 0, 1):
        for col in (-1, 0, 1):
            offs.append((1 - row, 1 - col))

    with tile.TileContext(nc) as tc:
        with (
            tc.tile_pool(name="const", bufs=1) as cpool,
            tc.tile_pool(name="accp", bufs=4) as apool,
            tc.tile_pool(name="dp", bufs=3) as dpool,
            tc.tile_pool(name="dap", bufs=8) as dapool,
        ):
            # neighbors first: tiny DMA gates nbneg -> first ACT abs
            nbt = cpool.tile([128, NT * 9], dt.float32, tag="nbt")
            nb_src = nb.ap().rearrange("(t nn) c k -> (nn c) t k", nn=4)
            nbt_v = nbt[:].rearrange("p (t k) -> p t k", t=NT)
            nc.sync.dma_start(nbt_v, nb_src)

            x_src = x.ap().rearrange("b c h w -> c b (h w)")

            # raw fp32 halves via HWDGE on sync+scalar queues
            raw = []
            for h in range(2):
                t = cpool.tile([128, HB * H * W], dt.float32, tag=f"raw{h}")
                tv = t[:].rearrange("p (b h w) -> p b (h w)", b=HB, h=H, w=W)
                for nn in range(4):
                    eng = nc.sync if nn < 2 else nc.scalar
                    eng.dma_start(
                        tv[nn * 32 : (nn + 1) * 32],
                        x_src[:, h * HB : (h + 1) * HB],
                    )
                raw.append(tv.rearrange("p b (h w) -> p b h w", h=H, w=W))

            # single padded fp16 tile, all 4 batches
            xpt = cpool.tile([128, B_LOC * PH * PW], dt.float16, tag="xpad")
            xp = xpt[:].rearrange("p (b h w) -> p b h w", b=B_LOC, h=PH, w=PW)
            nc.gpsimd.memset(xp[:, :, 0:PH:33, :], 0.0)
            nc.gpsimd.memset(xp[:, :, 1:33, 0:PW:33], 0.0)

            # negated neighbors for ACT bias: Abs(x + (-nb))
            nbneg = cpool.tile([128, NT * 9], dt.float32, tag="nbneg")
            nc.scalar.mul(nbneg[:], nbt[:], -1.0)

            def pad_copy(h):
                nc.scalar.copy(
                    xp[:, h * HB : (h + 1) * HB, 1 : 1 + H, 1 : 1 + W], raw[h]
                )

            out_v = out.ap().rearrange("b (t p) h w -> t p b (h w)", t=NT)
            # early chains drain via gpsimd (idle mid-kernel), late via the
            # two HWDGE queues so the tail isn't SWDGE-paced
            out_qs = [nc.gpsimd, nc.sync, nc.gpsimd, nc.sync, nc.scalar, nc.sync]
            qi = [0]

            def chain(nt, h, dve_ks, act_ks, split_out=False):
                bs = {0: (0, HB), 1: (HB, B_LOC), None: (0, B_LOC)}[h]
                nbf = bs[1] - bs[0]
                F = nbf * H * W
                acc = apool.tile([128, F], dt.float16, tag="acc")

                def win(k):
                    a, bcol = offs[k]
                    return xp[:, bs[0] : bs[1], a : a + H, bcol : bcol + W]

                nd = len(dve_ks)
                dtile = dpool.tile([128, nd * F], dt.float16, tag="dve")
                for i, k in enumerate(dve_ks):
                    d_v = dtile[:, i * F : (i + 1) * F].rearrange(
                        "p (b h w) -> p b h w", b=nbf, h=H, w=W
                    )
                    nc.vector.tensor_scalar(
                        d_v, win(k), nbt[:, nt * 9 + k : nt * 9 + k + 1],
                        None, Alu.subtract,
                    )
                nc.vector.tensor_scalar(
                    dtile[:].bitcast(dt.uint16), dtile[:].bitcast(dt.uint16),
                    0x7FFF, None, Alu.bitwise_and,
                )
                nc.vector.tensor_tensor(
                    acc[:], dtile[:, 0:F], dtile[:, F : 2 * F], Alu.max
                )
                for i in range(2, nd):
                    nc.vector.tensor_tensor(
                        acc[:], acc[:], dtile[:, i * F : (i + 1) * F], Alu.max
                    )
                for k in act_ks:
                    da = dapool.tile([128, F], dt.float16, tag="dact")
                    da_v = da[:].rearrange("p (b h w) -> p b h w", b=nbf, h=H, w=W)
                    nc.scalar.activation(
                        da_v, win(k), AF.Abs,
                        bias=nbneg[:, nt * 9 + k : nt * 9 + k + 1], scale=1.0,
                    )
                    nc.vector.tensor_tensor(acc[:], acc[:], da[:], Alu.max)

                acc_s = acc[:].rearrange("p (b s) -> p b s", b=nbf)
                if split_out:
                    for bi, eng in zip(range(nbf), (nc.sync, nc.scalar)):
                        eng.dma_start(
                            out_v[nt][:, bs[0] + bi : bs[0] + bi + 1],
                            acc_s[:, bi : bi + 1],
                        )
                else:
                    dst = out_v[nt] if h is None else out_v[nt][:, bs[0] : bs[1]]
                    out_qs[qi[0] % len(out_qs)].dma_start(dst, acc_s)
                    qi[0] += 1

            KD3 = (0, 2, 6)
            KD2 = (0, 6)
            KA6 = (4, 1, 3, 5, 7, 8)
            KA7 = (4, 2, 1, 3, 5, 7, 8)
            chain_specs = [
                (0, 0, KD3, KA6, False),
                (3, 0, KD3, KA6, False),
                (1, None, KD3, KA6, False),
                (2, None, KD2, KA7, False),
                (0, 1, KD3, KA6, False),
                (3, 1, KD3, KA6, True),
            ]
            pad_copy(0)
            for ci, (nt, h, kd, ka, so) in enumerate(chain_specs):
                chain(nt, h, kd, ka, split_out=so)
                if ci == 0:
                    pad_copy(1)

    nc.compile()
    return nc


def _get_module():
    if "nc" not in _module_cache:
        _module_cache["nc"] = _build_module()
    return _module_cache["nc"]


def _run(x, neighbors, trace=False):
    from concourse import bass_utils

    x = np.ascontiguousarray(x, dtype=np.float32)
    neighbors = np.ascontiguousarray(neighbors, dtype=np.float32)
    in_maps = []
    for core in range(NCORES):
        bg, ng = divmod(core, NG)
        in_maps.append(
            {
                "x": x[bg * B_LOC : (bg + 1) * B_LOC],
                "neighbors": neighbors[ng * N_LOC : (ng + 1) * N_LOC],
            }
        )
    res = bass_utils.run_bass_kernel_spmd(
        _get_module(), in_maps, core_ids=list(range(NCORES)), trace=trace
    )
    out = np.empty((B, NUM * C, H, W), dtype=np.float32)
    for core in range(NCORES):
        bg, ng = divmod(core, NG)
        out[bg * B_LOC : (bg + 1) * B_LOC, ng * N_LOC * C : (ng + 1) * N_LOC * C] = (
            res.results[core]["out"].astype(np.float32)
        )
    return out, res


def kernel(x, neighbors):
    out, _ = _run(x, neighbors, trace=False)
    return out
